# revision 28
# baseline (speedup 1.0000x reference)
"""Trainium2 Bass kernel for nn_CDA_attention (density-modulated attention).

Contract: kernel(**full_inputs) -> full output [8, 256, 64, 64] float32.
Data-parallel over batch: core b computes batch b.

Per-core computation (batch b, C=256, N=4096):
  - all GEMMs run in fp8e4 DoubleRow mode (K=256 per pass, 2x PE rate):
    projections q/k/vproj contract the channel dim in one pass; QK^T
    contracts channels; attn@V contracts key-chunk PAIRS.
  - x is converted to fp8 on the host (x8) for every GEMM input; the f32
    copy is DMA'd only for the final residual add.
  - gray/density chain: gray = mean_c x via DoubleRow matmul with a 1/C
    constant column; Laplacian -> conv(1->8) -> relu -> conv(8->1) ->
    sigmoid on DVE (image layout), producing skv[nk] = 1/temperature.
  - scores sT[nk, nq] = k8^T q8 with k8 = (k + bias) * skv pre-scaled;
    exp(score/16 - 2) is softmax-invariant-shifted to stay inside
    fp8e4's range and is computed on TWO engines in parallel: ACT (true
    exp) for 11/16 key pairs, DVE for 5/16 via a one-op Schraudolph
    bit-trick (fp8 bits = round(score*8*log2e/16 + const) as a
    saturating f32->uint8 convert bitcast to fp8).
  - attn@vproj (vproj = (Wout@Wv) x with a ones column -> row sums),
    rowsum-normalized, bf16-transposed back to [c, n] on the PE,
    + fused bias + residual.
"""

import os
import sys

sys.path.insert(0, "/opt/trn_rl_repo")

from contextlib import ExitStack

import ml_dtypes
import numpy as np

import concourse.bass as bass
import concourse.mybir as mybir
import concourse.tile as tile
from concourse import bacc, bass_utils
from concourse.masks import make_identity

B, C, HH, WW = 8, 256, 64, 64
N = HH * WW          # 4096
P = 128
CC = C // P          # 2 channel chunks
NQT = 512            # query tile (columns per QK^T matmul)
NQ_TILES = N // NQT  # 8
NKC = N // P         # 32 key chunks
NPAIR = NKC // 2     # 16 key-chunk pairs
NSUB = NQT // P      # 4 query sub-tiles per query tile

f32 = mybir.dt.float32
f32r = mybir.dt.float32r
bf16 = mybir.dt.bfloat16
f8 = mybir.dt.float8e4
u8 = mybir.dt.uint8
DR = mybir.MatmulPerfMode.DoubleRow
AF = mybir.ActivationFunctionType
ALU = mybir.AluOpType

# key pairs whose exp runs on DVE (Schraudolph) instead of ACT
DVE_EXP_PAIRS = tuple(
    int(t) for t in os.environ.get("KERNEL_DVE_PAIRS", "2,5,8,11,14").split(",")
    if t != "")
AV_LAG = int(os.environ.get("KERNEL_AV_LAG", "2"))
# fp8e4 Schraudolph constants: bits = score*SCH_A + SCH_B (f32->uint8,
# truncating convert => +0.5; -0.475 centers the mantissa-interp bias)
SCH_A = 8.0 * 1.4426950408889634 / 16.0
SCH_B = 56.0 - 2.0 * 8.0 * 1.4426950408889634 - 0.475 + 0.5

# tap order for 3x3 convs: center first so the first tap writes the full tile
TAPS = [(1, 1)] + [(ky, kx) for ky in range(3) for kx in range(3) if (ky, kx) != (1, 1)]


def _make_row_shifted(nc, pool, src, name):
    """Return {dy: AP} of row-shifted copies of src ([64, ...] SBUF tile):
    sh[+1][p] = src[p+1] (last row 0), sh[-1][p] = src[p-1] (first row 0).
    Compute engines need 32-aligned partition bases, DMA does not — so the
    row shift is done once by DMA into zeroed tiles and every conv tap then
    reads/writes full partition ranges."""
    shape = list(src.shape)
    p1 = pool.tile(shape, f32, name=f"{name}_p1")
    m1 = pool.tile(shape, f32, name=f"{name}_m1")
    nc.gpsimd.memset(p1[:], 0.0)
    nc.gpsimd.memset(m1[:], 0.0)
    nc.sync.dma_start(p1[0:63], src[1:64])
    nc.sync.dma_start(m1[1:64], src[0:63])
    return {0: src, 1: p1, -1: m1}


def build_kernel_body(tc, ctx, d):
    nc = tc.nc
    x_d, x8_d = d["x"], d["x8"]
    out_d, scr1, scr2 = d["out"], d["scr1"], d["scr2"]
    scr1_2d = scr1.rearrange("(a b) -> a b", a=1)

    const = ctx.enter_context(tc.tile_pool(name="const", bufs=1))
    big = ctx.enter_context(tc.tile_pool(name="big", bufs=1))
    ps_pool = ctx.enter_context(tc.tile_pool(name="ps", bufs=2, space="PSUM"))
    po_pool = ctx.enter_context(tc.tile_pool(name="po", bufs=2, space="PSUM"))
    fin_pool = ctx.enter_context(tc.tile_pool(name="fin", bufs=2))
    osb_pool = ctx.enter_context(tc.tile_pool(name="osb", bufs=2))
    rcp_pool = ctx.enter_context(tc.tile_pool(name="rcp", bufs=2))
    qt_pool = ctx.enter_context(tc.tile_pool(name="qt", bufs=2))
    repl_pool = ctx.enter_context(tc.tile_pool(name="repl", bufs=2))
    grow_pool = ctx.enter_context(tc.tile_pool(name="grow", bufs=2))

    # ---- persistent SBUF tiles ----
    XQ = N // 4
    x_parts = [big.tile([P, CC, XQ], f32, name=f"xp{t}") for t in range(4)]
    x8_parts = [big.tile([P, CC, XQ], f8, name=f"x8p{t}") for t in range(4)]

    def xsl(parts, start, size):
        t = start // XQ
        assert (start + size - 1) // XQ == t
        o = start - t * XQ
        return parts[t][:, :, o:o + size]

    k_sb = big.tile([P, CC, N], f32r)
    k8 = big.tile([P, CC, N], f8)
    vproj_sb = big.tile([P, NKC, C + 1], f8)
    exp_a = big.tile([P, NKC, NQT], f8)
    exp_b = big.tile([P, NKC, NQT], f8)
    wf8_sb = const.tile([P, CC, 3 * C], f8)   # [wqk | wvo] fused
    wqk8_sb = wf8_sb[:, :, 0:2 * C]
    wvo8_sb = wf8_sb[:, :, 2 * C:3 * C]
    qb6_sb = const.tile([P, 6], f32)          # [qkb(4) | bfin(2)] fused
    qkb_sb = qb6_sb[:, 0:4]
    bfin_sb = qb6_sb[:, 4:6]
    negb_sb = const.tile([P, 1], f32)      # -2.0 exp-bias column
    ones8_sb = const.tile([P, CC, 1], f8)  # 1/C column for the channel mean
    cw_sb = const.tile([64, 153], f32)        # fused conv weights
    w1b_sb = cw_sb[:, 0:72].rearrange("p (t o a) -> p t o a", o=8, a=1)
    w1bias_sb = cw_sb[:, 72:80].rearrange("p (o a) -> p o a", a=1)
    w2b_sb = cw_sb[:, 80:152].rearrange("p (t o a) -> p t o a", o=8, a=1)
    w2bias_sb = cw_sb[:, 152:153]
    ident_bf = const.tile([P, P], bf16)
    gray_img = const.tile([64, 64], f32)
    g_p1 = const.tile([64, 64], f32)
    g_m1 = const.tile([64, 64], f32)
    lap_t = const.tile([64, 64], f32)
    abs_t = const.tile([64, 1, 64], f32)
    h1_t = const.tile([64, 8, 64], f32)
    h1r_t = const.tile([64, 8, 64], f32)
    cacc_t = const.tile([64, 8, 64], f32)
    ctmp_t = const.tile([64, 8, 64], f32)
    dl_t = const.tile([64, 64], f32)
    sig_t = const.tile([64, 64], f32)
    skv_t = const.tile([64, 64], f32)

    # ---- input DMAs ----
    # x8 FIRST (the gray image needs all of it and gates the density chain);
    # fused weight tensors next (one SP dispatch each instead of ten); the
    # 4 MB f32 x (residual-only, needed late) is emitted just before the
    # attention loop so its transfers don't contend with x8.
    for t in range(4):
        for ci in range(CC):
            nc.sync.dma_start(
                x8_parts[t][:, ci, :], x8_d[ci * P:(ci + 1) * P, t * XQ:(t + 1) * XQ])
    nc.sync.dma_start(
        wf8_sb[:, :, :], d["wf8"].rearrange("(c p) w -> p c w", p=P))
    nc.sync.dma_start(qb6_sb[:, :], d["qb6"][:, :])
    nc.sync.dma_start(cw_sb[:, :], d["cw"][:, :])

    make_identity(nc, ident_bf)
    nc.gpsimd.memset(ones8_sb[:], 1.0 / C)
    nc.gpsimd.memset(negb_sb[:], -2.0)
    nc.gpsimd.memset(vproj_sb[:, :, C:C + 1], 1.0)    # ones column -> row sums
    nc.gpsimd.memset(g_p1[:], 0.0)
    nc.gpsimd.memset(g_m1[:], 0.0)

    # ---- gray + vproj interleaved per x8 quarter (PE tracks DMA arrival);
    # gray = mean_c x (plain fp8, M=1: a 1-wide DoubleRow weight load
    # violates the dual-fp8 ldweights ISA rules); PSUM -> SBUF, no DRAM hop
    for t in range(4):
        for nt in (2 * t, 2 * t + 1):
            pg = ps_pool.tile([1, NQT], f32, tag="ps")
            for ci in range(CC):
                nc.tensor.matmul(
                    pg[:, :], ones8_sb[:, ci, :],
                    xsl(x8_parts, nt * NQT, NQT)[:, ci, :],
                    start=(ci == 0), stop=(ci == CC - 1))
            grow = grow_pool.tile([1, NQT], f32)
            nc.vector.tensor_copy(grow[:, :], pg[:, :])
            nc.sync.dma_start(scr1_2d[:, nt * NQT:(nt + 1) * NQT], grow[:, :])
        for j2 in range(4 * t, 4 * t + 4):
            pv = po_pool.tile([P, 2, C], f32, tag="po")
            for u in range(2):
                nc.tensor.matmul(
                    pv[:, u, :], xsl(x8_parts, (2 * j2 + u) * P, P),
                    wvo8_sb[:, :, :], start=True, stop=True, perf_mode=DR)
            nc.scalar.activation(
                vproj_sb[:, 2 * j2:2 * j2 + 2, 0:C], pv[:, :, :], AF.Copy)
    # gray + the +-1-row-shifted variants read back from DRAM in parallel.
    # (Chaining SBUF->SBUF DMAs through gray_img raced on hardware — DMA
    # write completion vs a dependent DMA's read — so the image takes the
    # baseline's DRAM roundtrip, which is proven deterministic.)
    sh = scr1.rearrange("(h w) -> h w", w=64)
    nc.sync.dma_start(gray_img[:, :], sh[:, :])
    nc.sync.dma_start(g_p1[0:63, :], sh[1:64, :])
    nc.sync.dma_start(g_m1[1:64, :], sh[0:63, :])
    gvar = {0: gray_img, 1: g_p1, -1: g_m1}

    # ---- k projection (DoubleRow), nt PAIRS per m so the bias is shared ----
    for m in (2, 3):                     # c_out chunks k0, k1
        mm = m - 2
        for nt2 in range(NQ_TILES // 2):
            pk = ps_pool.tile([P, 2, NQT], f32, tag="ps")
            for u in range(2):
                nt = 2 * nt2 + u
                nc.tensor.matmul(
                    pk[:, u, :], wqk8_sb[:, :, m * P:(m + 1) * P],
                    xsl(x8_parts, nt * NQT, NQT), start=True, stop=True, perf_mode=DR)
            nc.scalar.activation(
                k_sb[:, mm, nt2 * 2 * NQT:(nt2 + 1) * 2 * NQT].rearrange(
                    "p (un n) -> p un n", un=2),
                pk[:, :, :], AF.Identity, bias=qkb_sb[:, m:m + 1])

    # ---- q projection for qtile 0 (DoubleRow; DVE-evicted) ----
    q_t0 = qt_pool.tile([P, CC, NQT], f8)
    for mm in range(CC):
        pq = ps_pool.tile([P, NQT], f32, tag="ps")
        nc.tensor.matmul(pq[:, :], wqk8_sb[:, :, mm * P:(mm + 1) * P],
                         xsl(x8_parts, 0, NQT), start=True, stop=True, perf_mode=DR)
        nc.vector.tensor_scalar(
            out=q_t0[:, mm, :], in0=pq[:, :],
            scalar1=qkb_sb[:, mm:mm + 1], scalar2=None, op0=ALU.add)

    # ---- density chain (image layout, DVE; ACT only for the sigmoid) ----
    # Laplacian: 4*g - up - down - left - right (zero SAME padding)
    nc.vector.tensor_scalar(
        out=lap_t[:, :], in0=gray_img[:, :], scalar1=4.0, scalar2=None, op0=ALU.mult)
    for dy in (1, -1):  # out[h] += -g[h+dy]
        nc.vector.scalar_tensor_tensor(
            out=lap_t[:, :], in0=gvar[dy][:, :], scalar=-1.0, in1=lap_t[:, :],
            op0=ALU.mult, op1=ALU.add)
    for dx in (1, -1):
        c0, c1 = max(0, -dx), WW - max(0, dx)
        dst = lap_t[:, c0:c1]
        nc.vector.scalar_tensor_tensor(
            out=dst, in0=gray_img[:, c0 + dx:c1 + dx], scalar=-1.0, in1=dst,
            op0=ALU.mult, op1=ALU.add)
    # abs on DVE: |x| = max(-x, x)
    nc.vector.scalar_tensor_tensor(
        out=abs_t[:, 0, :], in0=lap_t[:, :], scalar=-1.0, in1=lap_t[:, :],
        op0=ALU.mult, op1=ALU.max)

    # conv1: 1 -> 8 channels, all channels per tap in one wide op via
    # free-dim-broadcast APs
    avar = _make_row_shifted(nc, const, abs_t, "abs")

    def conv_taps(out_t, in_var, wpat):
        for i, (ky, kx) in enumerate(TAPS):
            dy, dx = ky - 1, kx - 1
            c0, c1 = max(0, -dx), WW - max(0, dx)
            L = c1 - c0
            src = in_var(dy, slice(c0 + dx, c1 + dx))
            w = wpat[:, ky * 3 + kx, :, :].broadcast_to([64, 8, L])
            if i == 0:
                assert (dy, dx) == (0, 0)
                nc.vector.tensor_mul(out_t[:, :, :], src, w)
            else:
                nc.vector.tensor_mul(ctmp_t[:, :, 0:L], src, w)
                nc.vector.tensor_add(
                    out_t[:, :, c0:c1], out_t[:, :, c0:c1], ctmp_t[:, :, 0:L])

    conv_taps(
        h1_t,
        lambda dy, cs: avar[dy][:, :, cs].broadcast_to(
            [64, 8, cs.stop - cs.start]),
        w1b_sb)
    # relu(h1 + bias) in two wide ops
    nc.vector.tensor_add(
        h1_t[:, :, :], h1_t[:, :, :], w1bias_sb.broadcast_to([64, 8, WW]))
    nc.vector.tensor_scalar(
        out=h1r_t[:, :, :], in0=h1_t[:, :, :], scalar1=0.0, scalar2=None,
        op0=ALU.max)

    # conv2: 8 -> 1 channel, then tree-reduce over ic; sigmoid with bias
    hvar = _make_row_shifted(nc, const, h1r_t, "h1r")
    conv_taps(cacc_t, lambda dy, cs: hvar[dy][:, :, cs], w2b_sb)
    nc.vector.tensor_add(cacc_t[:, 0:4, :], cacc_t[:, 0:4, :], cacc_t[:, 4:8, :])
    nc.vector.tensor_add(cacc_t[:, 0:2, :], cacc_t[:, 0:2, :], cacc_t[:, 2:4, :])
    nc.vector.tensor_add(dl_t[:, :], cacc_t[:, 0, :], cacc_t[:, 1, :])
    nc.scalar.activation(sig_t[:, :], dl_t[:, :], AF.Sigmoid, bias=w2bias_sb[:, 0:1])
    # skv = 1 / (3 - 2*sigmoid); the C^-0.5 score scale lives in the exp
    # activation (global scale=1/16) so k8 stays in fp8e4's normal range
    nc.scalar.activation(dl_t[:, :], sig_t[:, :], AF.Copy, bias=3.0, scale=-2.0)
    nc.vector.reciprocal(skv_t[:, :], dl_t[:, :])
    # skv -> DRAM, flat [4096] keyed by n = h*64+w (for partition broadcast)
    nc.sync.dma_start(scr2.rearrange("(h w) -> h w", w=64), skv_t[:, :])

    # ---- k8 = k * skv[nk] (fp8; per-key temperature folded into k) ----
    scr2_1 = scr2.rearrange("(a b) -> a b", a=1)
    for nt in range(NQ_TILES):
        sl = slice(nt * NQT, (nt + 1) * NQT)
        repl = repl_pool.tile([P, 1, NQT], f32)
        nc.sync.dma_start(
            repl[:, 0, :], scr2_1[0:1, sl].broadcast_to([P, NQT]))
        nc.vector.tensor_mul(
            k8[:, :, sl], k_sb[:, :, sl].bitcast(f32),
            repl[:, :, :].broadcast_to([P, CC, NQT]))

    # ---- f32 x for the residual add (needed from qtile 0's fin onward) ----
    for t in range(4):
        for ci in range(CC):
            nc.sync.dma_start(
                x_parts[t][:, ci, :], x_d[ci * P:(ci + 1) * P, t * XQ:(t + 1) * XQ])

    # ---- attention ----
    q_cur = q_t0
    for it in range(NQ_TILES):
        nq0 = it * NQT
        exp_sb = exp_a if it % 2 == 0 else exp_b
        q_t = q_cur

        # two paired attn@V accumulators [P, 2, NQT] (cols 0:C+1 used)
        pos = [po_pool.tile([P, 2, NQT], f32, tag="po", name=f"po{s2}")
               for s2 in range(NSUB // 2)]

        def attnv_pair(jj):
            for s in range(NSUB):
                nc.tensor.matmul(
                    pos[s // 2][:, s % 2, 0:C + 1],
                    exp_sb[:, 2 * jj:2 * jj + 2, s * P:(s + 1) * P],
                    vproj_sb[:, 2 * jj:2 * jj + 2, :],
                    start=(jj == 0), stop=(jj == NPAIR - 1),
                    perf_mode=DR)

        q_nxt = None
        for jj in range(NPAIR):
            ps2 = ps_pool.tile([P, 2, NQT], f32, tag="ps")
            for u in range(2):
                j = 2 * jj + u
                nc.tensor.matmul(
                    ps2[:, u, :], k8[:, :, j * P:(j + 1) * P], q_t[:, :, :],
                    start=True, stop=True, perf_mode=DR)
            if jj in DVE_EXP_PAIRS:
                # Schraudolph fp8 exp on DVE: bits = score*A + B, saturating
                # f32->uint8 convert, bitcast to fp8e4
                nc.vector.tensor_scalar(
                    out=exp_sb[:, 2 * jj:2 * jj + 2, :].bitcast(u8),
                    in0=ps2[:, :, :], scalar1=SCH_A, scalar2=SCH_B,
                    op0=ALU.mult, op1=ALU.add)
            else:
                nc.scalar.activation(
                    exp_sb[:, 2 * jj:2 * jj + 2, :], ps2[:, :, :], AF.Exp,
                    bias=negb_sb[:, 0:1], scale=float(C) ** -0.5)
            if jj >= AV_LAG:
                # lag the attn@V consumption behind the scores: AV(jj-lag)'s
                # exp finished before QK(jj) could even get its psum slot,
                # so the in-order PE never stalls on an exp handoff and the
                # ACT/DVE exps of adjacent pairs overlap.
                attnv_pair(jj - AV_LAG)
            if jj == 4 and it + 1 < NQ_TILES:
                # hoisted q projection for the NEXT qtile: PE has slack here
                # and the psum pool rotation has a free slot
                q_nxt = qt_pool.tile([P, CC, NQT], f8)
                for mm in range(CC):
                    pq = ps_pool.tile([P, NQT], f32, tag="ps")
                    nc.tensor.matmul(
                        pq[:, :], wqk8_sb[:, :, mm * P:(mm + 1) * P],
                        xsl(x8_parts, (it + 1) * NQT, NQT),
                        start=True, stop=True, perf_mode=DR)
                    nc.vector.tensor_scalar(
                        out=q_nxt[:, mm, :], in0=pq[:, :],
                        scalar1=qkb_sb[:, mm:mm + 1], scalar2=None, op0=ALU.add)
        for jj in range(NPAIR - AV_LAG, NPAIR):
            attnv_pair(jj)
        q_cur = q_nxt

        # softmax normalization + transpose back to [c, n] + bias + residual
        rcp = rcp_pool.tile([P, NSUB, 1], f32)
        osb = osb_pool.tile([P, NSUB, C], bf16)
        for s2 in range(NSUB // 2):
            nc.vector.reciprocal(
                rcp[:, 2 * s2:2 * s2 + 2, :], pos[s2][:, :, C:C + 1])
        for s in range(NSUB):
            nc.vector.tensor_scalar(
                out=osb[:, s, :], in0=pos[s // 2][:, s % 2, 0:C],
                scalar1=rcp[:, s:s + 1, 0], scalar2=None, op0=ALU.mult)
        fin = fin_pool.tile([P, CC, NQT], f32)
        for ci in range(CC):
            pt = po_pool.tile([P, NQT], bf16, tag="po", name="pt")
            for s in range(NSUB):
                nc.tensor.transpose(
                    pt[:, s * P:(s + 1) * P], osb[:, s, ci * P:(ci + 1) * P],
                    ident_bf[:, :])
            nc.vector.scalar_tensor_tensor(
                out=fin[:, ci, :], in0=pt[:, :],
                scalar=bfin_sb[:, ci:ci + 1],
                in1=xsl(x_parts, nq0, NQT)[:, ci, :],
                op0=ALU.add, op1=ALU.add)
        for ci in range(CC):
            nc.sync.dma_start(out_d[ci * P:(ci + 1) * P, nq0:nq0 + NQT], fin[:, ci, :])


def build_nc():
    nc = bacc.Bacc("TRN2", target_bir_lowering=False, debug=False)
    d = {}
    def inp(name, shape, dt=f32):
        d[name] = nc.dram_tensor(name, shape, dt, kind="ExternalInput").ap()
    inp("x", (C, N))
    inp("x8", (C, N), f8)
    inp("wf8", (C, 3 * C), f8)
    inp("qb6", (P, 6))
    inp("cw", (64, 153))
    d["out"] = nc.dram_tensor("out", (C, N), f32, kind="ExternalOutput").ap()
    d["scr1"] = nc.dram_tensor("scr1", (N,), f32, kind="Internal").ap()
    d["scr2"] = nc.dram_tensor("scr2", (N,), f32, kind="Internal").ap()

    with tile.TileContext(nc) as tc, ExitStack() as ctx:
        build_kernel_body(tc, ctx, d)
    nc.compile()
    return nc


def host_inputs(x, qkv_w, qkv_b, out_w, out_b, d1_w, d1_b, d2_w, d2_b):
    f = np.float32
    f8np = ml_dtypes.float8_e4m3
    x = np.asarray(x, f)
    wq = np.asarray(qkv_w, f)[:, :, 0, 0]          # [768, 256]
    qkv_b = np.asarray(qkv_b, f)
    wout = np.asarray(out_w, f)[:, :, 0, 0]        # [256, 256]
    out_b = np.asarray(out_b, f)
    wf8 = np.concatenate(
        [wq[0:2 * C].T, (wout @ wq[2 * C:3 * C]).T], axis=1)
    qb6 = np.concatenate(
        [qkv_b[0:2 * C].reshape(4, P).T,
         (wout @ qkv_b[2 * C:3 * C] + out_b).reshape(2, P).T], axis=1)
    # tap-major [t*8 + ch] weight patterns for the wide conv ops, fused
    cw1 = np.concatenate(
        [np.asarray(d1_w, f).reshape(8, 9).T.reshape(72),
         np.asarray(d1_b, f).reshape(8),
         np.asarray(d2_w, f).reshape(8, 9).T.reshape(72),
         np.asarray(d2_b, f).reshape(1)])
    shared = {
        "wf8": np.ascontiguousarray(wf8).astype(f8np),
        "qb6": np.ascontiguousarray(qb6, dtype=f),
        "cw": np.tile(cw1.reshape(1, 153), (64, 1)).astype(f),
    }
    xs = x.reshape(B, C, N)
    return [dict(x=np.ascontiguousarray(xs[b]),
                 x8=np.ascontiguousarray(xs[b]).astype(f8np), **shared)
            for b in range(B)]


_NC_CACHE = {}


def _get_nc():
    if "nc" not in _NC_CACHE:
        _NC_CACHE["nc"] = build_nc()
    return _NC_CACHE["nc"]


def kernel(x, qkv_w, qkv_b, out_w, out_b, d1_w, d1_b, d2_w, d2_b):
    in_maps = host_inputs(x, qkv_w, qkv_b, out_w, out_b, d1_w, d1_b, d2_w, d2_b)
    nc = _get_nc()
    trace = bool(int(os.environ.get("KERNEL_TRACE", "0")))
    res = bass_utils.run_bass_kernel_spmd(
        nc, in_maps, core_ids=list(range(B)), trace=trace)
    _NC_CACHE["last_results"] = res
    out = np.stack([res.results[b]["out"] for b in range(B)])
    return np.ascontiguousarray(out.reshape(B, C, HH, WW).astype(np.float32))
